# revision 26
# baseline (speedup 1.0000x reference)
"""Trainium2 Bass kernel for nn_EntailmentSelfAttention (8-core data parallel).

Problem (per batch element n, sentence s):
  q/k/v head projections (shared per-head weights), energy = q @ k.T per head,
  query-position masking, softmax over the QUERY axis, out = attn @ v,
  fc_out: out @ Wo.T + bo.

Mapping (one (n) per NeuronCore; S=2 sentences iterated inside):
  - All tensors kept "transposed" on-chip: head-dim/embed-dim on partitions,
    sequence on the free axis, so the softmax (over queries) reduces along the
    free axis.
  - The V projection is folded into fc_out on the host:
      out = concat_h((attn_h @ xv_h) @ Wv.T) @ Wo.T = concat_h(attn_h @ xv_h) @ Wcomb
  - The query mask enters the energy matmul as a 65th contraction row:
      kT_ext row64 = 1, qT_ext row64 = -3000 * (1 - mask_q); after the 1/sqrt(L)
      softmax scale this is -132.6 -> exp underflows to 0 exactly, matching the
      reference's -1e20 semantics.
  - The K projection is folded into the energy matmul on the host:
      energy^T = xk^T (Wk^T Wq) xq, so raw (transposed) keys from DMA are the
      stationary operand directly.
  - Softmax denominators come from the Exp activation's accum_out (3/8) and
    vector-engine reduces (5/8, load balance); the 1/rowsum normalization is
    folded into a per-l-row scale of xv before the attn @ xv matmul.
  - Masked query positions are dropped on the host (compaction to QP columns);
    their output rows are exactly the fc bias, filled host-side.
"""

import math

import numpy as np

import concourse.bass as bass
import concourse.tile as tile
from concourse import bacc, mybir
from concourse import bass_utils

# problem shapes (hardcoded per the harness contract)
N, S, L, E, H = 8, 2, 512, 1024, 16
D = E // H  # 64
DX = D + 1  # extended head dim (projection + mask/ones row)
P = 128
NCORES = 8
LC = L // P  # 4 l-chunks
BMASK = 3000.0
QP_MIN = 256  # min compacted query length (keeps matmul free dims efficient)
SCALE = 1.0 / math.sqrt(float(L))

F32 = mybir.dt.float32
BF16 = mybir.dt.bfloat16
# matmul compute dtype. bf16: 1 cyc/row, halves input DMA bytes, and (unlike
# float32r) supports PSUM dst partitions 64:128 for the paired attn@v banks.
# float32r also works (rel err ~2e-3 -> ~2e-4) at ~+15us.
MM_DT = mybir.dt.bfloat16


DT_MM = MM_DT  # dtype for all matmul-operand tiles / DRAM tensors


def build_kernel_body(tc, outs, ins, QP):
    nc = tc.nc

    def _c(ap):
        # sim path: run_kernel declares DRAM as plain fp32; view as DT_MM
        return ap if ap.dtype == DT_MM else ap.bitcast(DT_MM)

    xq, xk, xv = _c(ins["xq"]), _c(ins["xk"]), ins["xv"]
    wq, wcomb, bo = _c(ins["wq"]), _c(ins["wcomb"]), ins["bo"]
    npad = ins["npad"]
    outT = outs["outT"]

    import contextlib

    with contextlib.ExitStack() as ctx:
        ek = ctx.enter_context
        consts = ek(tc.tile_pool(name="consts", bufs=1))
        stream = ek(tc.tile_pool(name="stream", bufs=4))
        qkpool = ek(tc.tile_pool(name="qk", bufs=2))
        xvpool = ek(tc.tile_pool(name="xv", bufs=1))
        xvspool = ek(tc.tile_pool(name="xvs", bufs=4))
        attnpool = ek(tc.tile_pool(name="attn", bufs=10))
        sumpool = ek(tc.tile_pool(name="sums", bufs=8))
        ztpool = ek(tc.tile_pool(name="zt", bufs=1))
        outpool = ek(tc.tile_pool(name="out", bufs=3))
        pp_pf = ek(tc.tile_pool(name="pp_pf", bufs=2, space="PSUM"))
        pp_e = ek(tc.tile_pool(name="pp_e", bufs=4, space="PSUM"))
        pp_z = ek(tc.tile_pool(name="pp_z", bufs=1, space="PSUM"))

        # constants (wq holds the fused-projection lhsT: see host_prepare)
        wq_sb = consts.tile([D, D], DT_MM, tag="wq")
        nc.sync.dma_start(wq_sb[:], wq[:])
        npad_sb = consts.tile([P, S], F32, tag="npad")
        for s_ in range(S):
            nc.sync.dma_start(npad_sb[:, s_:s_ + 1], npad[s_])

        GH = 4  # heads per group (PSUM: one z bank per head pair)
        ZT_done = {}
        wcomb_sb = consts.tile([P, E // P, E], DT_MM, tag="wcomb")
        bo_sb = consts.tile([P, E // P], F32, tag="bo")
        wcomb_loaded = [False]

        def load_wcomb():
            # emitted after the first attention group's DMAs so the 4MB
            # transfer doesn't delay kernel start
            nc.sync.dma_start(
                wcomb_sb[:], wcomb.rearrange("(eo p) j -> p eo j", p=P))
            nc.sync.dma_start(bo_sb[:], bo.rearrange("(jo p) -> p jo", p=P))
            wcomb_loaded[0] = True

        def emit_fc_jt(s, ZT, jt):
            fp = pp_pf.tile([P, QP], F32, tag="pf", name=f"fp_{s}_{jt}")
            for eo in range(E // P):
                nc.tensor.matmul(
                    fp[:],
                    wcomb_sb[:, eo, jt * P:(jt + 1) * P],
                    ZT[:, eo, :],
                    start=(eo == 0),
                    stop=(eo == E // P - 1),
                )
            ot = outpool.tile([P, QP], F32, tag="ot", name=f"ot_{s}_{jt}")
            nc.scalar.activation(
                ot[:], fp[:], mybir.ActivationFunctionType.Identity,
                bias=bo_sb[:, jt:jt + 1])
            nc.sync.dma_start(outT[s, jt * P:(jt + 1) * P, :], ot[:])

        for s in range(S):
            # values for this sentence: [p, lc, e], l = lc*128 + p
            xv_sb = xvpool.tile([P, LC, E], BF16, tag=f"xv{s % 2}")
            nc.sync.dma_start(xv_sb[:], xv[s].rearrange("(lo p) e -> p lo e", p=P))

            ZT = ztpool.tile([P, E // P, QP], DT_MM, tag=f"zt{s % 2}", name=f"zt_{s}")
            for g in range(H // GH):
                h0 = g * GH
                # projections: per head qT_ext/kT_ext; group q-projs then
                # k-projs so the stationary weight reloads only once.
                qes = []
                xq_g = stream.tile([D, GH, QP], DT_MM, tag="xq_g")
                nc.sync.dma_start(xq_g[:], xq[s, g])
                # raw transposed keys, head pairs stacked on partitions, act
                # directly as the row-tiled energy stationary operand
                xk_g = stream.tile([P, GH // 2, L], DT_MM, tag="xk_g")
                nc.sync.dma_start(xk_g[:], xk[s, 2 * g:2 * g + 2].rearrange(
                    "t d l -> d t l"))
                # projected queries, head pairs stacked on partitions 0:64/64:128
                for p_ in range(GH // 2):
                    pq = pp_pf.tile([P, QP], F32, tag="pf", name="pq")
                    nc.tensor.matmul(pq[0:D, :], wq_sb[:], xq_g[:, 2 * p_],
                                     start=True, stop=True, skip_group_check=True)
                    nc.tensor.matmul(pq[D:P, :], wq_sb[:], xq_g[:, 2 * p_ + 1],
                                     start=True, stop=True,
                                     tile_position=(0, D), skip_group_check=True)
                    qe = qkpool.tile([P, QP], DT_MM, tag=f"qe{p_}", name=f"qe_{s}_{g}_{p_}")
                    nc.vector.tensor_copy(qe[:], pq[:])
                    qes.append(qe)

                # one z psum bank per head PAIR: head A -> partitions 0:64,
                # head B -> partitions 64:128 (separate accumulation groups).
                zps = [
                    pp_z.tile([P, QP], F32, tag=f"z{p_}", name=f"zp_{s}_{g}_{p_}")
                    for p_ in range(GH // 2)
                ]
                for c in range(LC):
                    rsum = sumpool.tile([P, GH], F32, tag="rsum")
                    ats = []
                    for i in range(GH):
                        ep = pp_e.tile([P, QP], F32, tag="energy", name="ep")
                        lo = (i % 2) * D
                        nc.tensor.matmul(
                            ep[:],
                            xk_g[lo:lo + D, i // 2, c * P:(c + 1) * P],
                            qes[i // 2][lo:lo + D, :],
                            start=True,
                            stop=True,
                        )
                        at = attnpool.tile([P, QP], BF16, tag="at", name="at")
                        if (c * GH + i) % 8 < 3:
                            # rowsum on the scalar engine (fused accumulate)
                            nc.scalar.activation(
                                at[:],
                                ep[:],
                                mybir.ActivationFunctionType.Exp,
                                scale=SCALE,
                                accum_out=rsum[:, i:i + 1],
                            )
                        else:
                            # rowsum on the vector engine (load balance)
                            nc.scalar.activation(
                                at[:],
                                ep[:],
                                mybir.ActivationFunctionType.Exp,
                                scale=SCALE,
                            )
                            nc.vector.tensor_reduce(
                                rsum[:, i:i + 1],
                                at[:],
                                axis=mybir.AxisListType.X,
                                op=mybir.AluOpType.add,
                            )
                        ats.append(at)
                    nc.vector.tensor_scalar(
                        rsum[:], rsum[:], npad_sb[:, s:s + 1], None,
                        mybir.AluOpType.subtract)
                    recip = sumpool.tile([P, GH], F32, tag="recip")
                    nc.vector.reciprocal(recip[:], rsum[:])
                    # xvs[p, i, d] = xv[p, c, (h0+i)*64+d] * recip[p, i]
                    xvs = xvspool.tile([P, GH, D], BF16, tag="xvs")
                    nc.vector.tensor_tensor(
                        xvs[:],
                        xv_sb[:, c, h0 * D:(h0 + GH) * D].rearrange(
                            "p (h d) -> p h d", d=D),
                        recip[:, :, None].to_broadcast((P, GH, D)),
                        mybir.AluOpType.mult,
                    )
                    for i in range(GH):
                        zp = zps[i // 2]
                        lo = (i % 2) * D
                        nc.tensor.matmul(
                            zp[lo:lo + D, :],
                            xvs[:, i],
                            ats[i][:],
                            start=(c == 0),
                            stop=(c == LC - 1),
                            skip_group_check=True,
                        )
                for p_ in range(GH // 2):
                    nc.vector.tensor_copy(ZT[:, g * (GH // 2) + p_, :], zps[p_][:])
                if not wcomb_loaded[0]:
                    load_wcomb()

            for jt in range(E // P):
                emit_fc_jt(s, ZT, jt)


def host_prepare(values, keys, query, mask, Wv, Wk, Wq, Wo, bo):
    """Host-side sharding + layout + query compaction.

    Returns (in_maps, QP, order, cnt, bo_np). Masked query positions are
    dropped entirely (their output is just bo); the surviving queries are
    compacted to the front and padded to QP columns. Pad columns carry a
    -BMASK bias row so their exp is exactly 0 (excluded from denominators).
    """
    values = np.ascontiguousarray(np.asarray(values, dtype=np.float32))
    keys = np.asarray(keys, dtype=np.float32)
    query = np.asarray(query, dtype=np.float32)
    mask = np.asarray(mask)
    Wv = np.asarray(Wv, dtype=np.float32)
    Wk = np.asarray(Wk, dtype=np.float32)
    Wq = np.asarray(Wq, dtype=np.float32)
    Wo = np.asarray(Wo, dtype=np.float32)
    bo_np = np.ascontiguousarray(np.asarray(bo, dtype=np.float32))

    keep = mask[:, :, :, 0] != 0  # (N, S, L) True = query position survives
    cnt = keep.sum(-1)  # (N, S)
    QP = int(np.ceil(max(int(cnt.max()), 1) / 64) * 64)
    QP = max(QP, QP_MIN)
    QP = min(QP, L)
    # stable partition: surviving query indices first
    order = np.argsort(~keep, axis=-1, kind="stable")  # (N, S, L)

    qT = query.transpose(0, 1, 3, 2).reshape(N, S, H, D, L)
    kT = keys.transpose(0, 1, 3, 2).reshape(N, S, H, D, L)
    npad = (QP - cnt).astype(np.float32)  # (N, S) pad cols contribute exp(0)=1

    # gather+pad queries: (N, S, H, D, QP)
    gidx = order[:, :, :QP]  # (N, S, QP)
    qTc = np.take_along_axis(
        qT, gidx[:, :, None, None, :].repeat(H, 2).repeat(D, 3), axis=4)
    pad = np.arange(QP)[None, None, :] >= cnt[:, :, None]  # (N, S, QP)
    qTc[pad[:, :, None, None, :].repeat(H, 2).repeat(D, 3)] = 0.0
    qb_row = np.where(pad, np.float32(-BMASK), np.float32(0.0)).astype(np.float32)
    GH = 4
    # (N,S,H,D,QP) -> (N,S,H//GH,D,GH,QP), no bias row (pads handled via npad)
    xq = np.ascontiguousarray(
        qTc.reshape(N, S, H // GH, GH, D, QP).transpose(0, 1, 2, 4, 3, 5))
    # keys: head PAIRS stacked on partitions [2*D=128, ...] for row-tiled energy
    xk = np.ascontiguousarray(
        kT.reshape(N, S, H // 2, 2, D, L).transpose(0, 1, 2, 3, 4, 5)
        .reshape(N, S, H // 2, 2 * D, L))

    # fused projection: energyT = xk^T (Wk^T Wq) xq -> yq = (Wk^T Wq) @ xqT,
    # lhsT[dj, di] = (Wq^T Wk)[dj, di]
    wq_ext = np.ascontiguousarray((Wq.T @ Wk).astype(np.float32))
    wk_ext = np.eye(D, dtype=np.float32).astype(__import__("ml_dtypes").bfloat16)  # unused placeholder

    wcomb = np.zeros((E, E), np.float32)
    for h in range(H):
        wcomb[h * D:(h + 1) * D, :] = Wv.T @ Wo[:, h * D:(h + 1) * D].T
    wcomb = np.ascontiguousarray(wcomb)

    import ml_dtypes
    bf = ml_dtypes.bfloat16
    values_bf = np.ascontiguousarray(values.astype(bf))
    xq = np.ascontiguousarray(xq.astype(bf))
    xk = np.ascontiguousarray(xk.astype(bf))
    wq_ext = wq_ext.astype(bf)
    wk_ext = wk_ext.astype(bf)
    wcomb = np.ascontiguousarray(wcomb.astype(bf))
    shared = {"wq": wq_ext, "wk": wk_ext, "wcomb": wcomb, "bo": bo_np}
    in_maps = []
    for n in range(NCORES):
        m = {"xq": xq[n], "xk": xk[n], "xv": values_bf[n],
             "npad": np.ascontiguousarray(
                 np.broadcast_to(npad[n][:, None, None], (S, P, 1)).astype(np.float32))}
        m.update(shared)
        in_maps.append(m)
    return in_maps, QP, order, cnt, bo_np


_NC_CACHE = {}


def _get_program(QP):
    nc = _NC_CACHE.get(QP)
    if nc is not None:
        return nc
    nc = bacc.Bacc("TRN2", target_bir_lowering=False, debug=False,
                   num_devices=NCORES)
    ins = {
        "xq": nc.dram_tensor("xq", (S, H // 4, D, 4, QP), DT_MM, kind="ExternalInput").ap(),
        "xk": nc.dram_tensor("xk", (S, H // 2, 2 * D, L), DT_MM, kind="ExternalInput").ap(),
        "xv": nc.dram_tensor("xv", (S, L, E), BF16, kind="ExternalInput").ap(),
        "wq": nc.dram_tensor("wq", (D, D), DT_MM, kind="ExternalInput").ap(),
        "npad": nc.dram_tensor("npad", (S, P, 1), F32, kind="ExternalInput").ap(),
        "wk": nc.dram_tensor("wk", (D, D), DT_MM, kind="ExternalInput").ap(),
        "wcomb": nc.dram_tensor("wcomb", (E, E), DT_MM, kind="ExternalInput").ap(),
        "bo": nc.dram_tensor("bo", (E,), F32, kind="ExternalInput").ap(),
    }
    outs = {
        "outT": nc.dram_tensor("outT", (S, E, QP), F32, kind="ExternalOutput").ap(),
    }
    with tile.TileContext(nc) as tc:
        build_kernel_body(tc, outs, ins, QP)
    nc.compile()
    _NC_CACHE[QP] = nc
    return nc


def run(inputs: dict, trace: bool = False):
    """Run on 8 cores; returns (full_output, BassKernelResults)."""
    in_maps, QP, order, cnt, bo_np = host_prepare(**inputs)
    nc = _get_program(QP)
    res = bass_utils.run_bass_kernel_spmd(
        nc, in_maps, core_ids=list(range(NCORES)), trace=trace,
    )
    out = np.empty((N, S, L, E), np.float32)
    out[:] = bo_np  # masked query rows: attention output is 0, fc adds bo
    for n in range(NCORES):
        oT = res.results[n]["outT"]  # (S, E, QP)
        for s in range(S):
            c = int(cnt[n, s])
            if c:
                out[n, s, order[n, s, :c], :] = oT[s, :, :c].T
    return out, res


def kernel(**inputs) -> np.ndarray:
    out, _ = run(inputs, trace=False)
    return out
